# revision 19
# baseline (speedup 1.0000x reference)
"""Trainium2 kernel for nn_MultiHeadCrossAttention_28063316313030.

Math: with seq_len == 1, softmax over a size-1 axis is identically 1, so
attention(Q,K,V) == V and W_Q/W_K are dead code.  The whole module collapses to

    out = LN(x1 @ A) + LN(x2 @ A),   A = W_V.T @ W_fc.T   (1024 x 1024)

where LN is LayerNorm over the last dim (gamma/beta fold in on host).

Key algebraic trick: the host CENTERS A's columns over the output dim
(Ac = A - rowmean_o(A), in float64).  Then every row of y' = x @ Ac has
exactly zero mean, so LayerNorm collapses to a pure scale:

    LN(y) = y' * r,   r = 1/sqrt(mean_o(y'^2) + eps)

No mean subtraction on device at all -- the epilogue is bn_stats/bn_aggr ->
sqrt/recip -> one multiply (stream 0 on ACT) and one fused multiply-add
(stream 1 on DVE scalar_tensor_tensor, which also folds the cross-stream
add).  Measured rel err ~4e-3 vs the 2e-2 gate.

Distribution: pure data parallel over the batch dim across 8 NeuronCores.
Host pre-tiles x1/x2 C-major so the TensorE contraction dim lands on SBUF
partitions with fully contiguous DMA runs.  Everything on the PE path is
bf16; LN stats stay in f32 (PSUM + stats).

Schedule per core (2048 rows per stream, 16 b-tiles x 2 streams):
  DMA issues are emitted FIRST on each engine: A k-tile 0 is split into two
  512-col halves (gpsimd/scalar) so the first steady matmul's dependency is
  128KB instead of 256KB; remaining A k-tiles alternate gpsimd/scalar.
  x b-tile 0 rides sync; b-tiles 1-2 queue behind A on gpsimd; later x
  tiles prefetch 2 deep on sync, WAR-paced by the 3-buffer x pool.
  PE warmup: 5 matmuls on a memset tile (no DMA dependency) bridge the gap
  from engine-preamble end (~7us) to first-data (~9.5us) and start the HAM
  clock-ramp window early, so steady matmuls run at full clock almost
  immediately.
  Steady b-tiles run k-major across the 4 (stream, half) PSUM banks; the
  fused epilogue per b-tile is stats -> r, then n0 = ps0*r0 (ACT) and
  out = ps1*r1 + n0 (one DVE pass, bf16 out), store on gpsimd.
  Endgame: the last two b-tiles' stream-0 halves are HOISTED: the tail runs
  b14s0, b15s0, b14s1, b15s1 as four single-stream units, each unit's
  epilogue hiding under the next unit's matmuls.  b15s1 runs h-outer with
  its second 512-col half accumulated as two N=256 column groups, so after
  the very last matmul only a 256-col bn_stats -> aggr -> sqrt -> recip ->
  fused scale-add -> split store remains (~3us).
"""

import sys

sys.path.insert(0, "/opt/trn_rl_repo")

import numpy as np

B, C, OUT = 16384, 1024, 1024
EPS = 1e-5
NCORES = 8
R = B // NCORES  # rows per core per stream
P = 128
KT = C // P  # contraction tiles
BT = R // P  # row tiles per core
NH = OUT // 512  # psum bank halves per row tile
N_WARMUP = 10

_cache = {}


def _build(mm_dtype_name: str, out_dtype_name: str):
    import concourse.bacc as bacc
    import concourse.bass as bass
    import concourse.mybir as mybir
    from concourse.tile import TileContext

    f32 = mybir.dt.float32
    bf16 = mybir.dt.bfloat16
    mmdt = getattr(mybir.dt, mm_dtype_name)
    outdt = getattr(mybir.dt, out_dtype_name)
    AF = mybir.ActivationFunctionType
    ALU = mybir.AluOpType

    nc = bacc.Bacc("TRN2", target_bir_lowering=False, debug=False, num_devices=NCORES)

    # host-pretiled: [ki, bt, ko, bi]
    x1p = nc.declare_dram_parameter("x1p", [P, BT, KT, P], mmdt, isOutput=False)
    x2p = nc.declare_dram_parameter("x2p", [P, BT, KT, P], mmdt, isOutput=False)
    # host-pretiled: [ki, ko, o]
    a_d = nc.declare_dram_parameter("a", [P, KT, OUT], mmdt, isOutput=False)
    y_d = nc.declare_dram_parameter("y", [R, OUT], outdt, isOutput=True)

    with TileContext(nc) as tc:
        with (
            tc.tile_pool(name="singles", bufs=1) as singles,
            tc.tile_pool(name="xs", bufs=3) as xpool,
            tc.tile_pool(name="ns", bufs=3) as npool,
            tc.tile_pool(name="outs", bufs=3) as opool,
            tc.tile_pool(name="stats", bufs=4) as stats,
            tc.tile_pool(name="psum", bufs=2, space="PSUM") as psum,
        ):
            # --- A k-tiles first (the critical stream).  k0 split into two
            # 512-col halves so the first matmul's dependency is half-size;
            # the rest alternate gpsimd (even) / scalar (odd).
            a0h = []
            for h in range(NH):
                t = singles.tile([P, 512], mmdt, tag=f"a0h{h}", name=f"a0h{h}")
                eng = nc.gpsimd if h == 0 else nc.scalar
                eng.dma_start(t[:], a_d[:, 0, h * 512 : (h + 1) * 512])
                a0h.append(t)
            a_sb = [None] * KT
            for k in range(1, KT):
                t = singles.tile([P, OUT], mmdt, tag=f"a{k}", name=f"a{k}")
                eng = nc.gpsimd if k % 2 == 0 else nc.scalar
                eng.dma_start(t[:], a_d[:, k, :])
                a_sb[k] = t

            def a_rhs(k, h):
                if k == 0:
                    return a0h[h][:]
                return a_sb[k][:, h * 512 : (h + 1) * 512]

            # x b-tile 0 on sync (needed first), split into half-k chunks so
            # the first steady matmul's gate is a 128KB transfer; b-tiles 1-2
            # queue behind A on gpsimd.
            xt_pre = {}
            KH = KT // 2

            def issue_x(bt, eng):
                for s, xp in enumerate((x1p, x2p)):
                    t = xpool.tile(
                        [P, KT, P], mmdt, tag=f"xt{s}", name=f"xt{bt}_{s}"
                    )
                    eng.dma_start(t[:], xp[:, bt])
                    xt_pre[(bt, s)] = t

            for s, xp in enumerate((x1p, x2p)):
                ta = singles.tile([P, KH, P], mmdt, tag=f"xta{s}", name=f"xta0_{s}")
                tb = singles.tile([P, KH, P], mmdt, tag=f"xtb{s}", name=f"xtb0_{s}")
                nc.sync.dma_start(ta[:], xp[:, 0, 0:KH])
                nc.sync.dma_start(tb[:], xp[:, 0, KH:KT])
                xt_pre[(0, s)] = (ta, tb)
            issue_x(1, nc.gpsimd)
            issue_x(2, nc.gpsimd)

            def xk(t, k):
                """k-slice of an x tile (handles b0's split half-k tiles)."""
                if isinstance(t, tuple):
                    return t[0][:, k, :] if k < KH else t[1][:, k - KH, :]
                return t[:, k, :]

            # --- PE warmup on a memset tile: no DMA dependency.  Emitted
            # after the DMA issues so it doesn't delay descriptor rings.
            warm_sb = singles.tile([P, 512], bf16)
            nc.vector.memset(warm_sb, 0.5)
            warm_ps = psum.tile([P, 512], f32, tag="ps11")
            for w in range(N_WARMUP):
                lo = 128 * (w % 2)
                nc.tensor.matmul(
                    warm_ps[:, 0:256], lhsT=warm_sb[:, lo : lo + P],
                    rhs=warm_sb[:, 0:256], start=True, stop=True,
                )

            eps_sb = singles.tile([P, 1], f32)
            nc.vector.memset(eps_sb, EPS)

            def stream_stats(bt, s, ps_tiles):
                """bn stats -> r = 1/sqrt(var+eps) for one stream (A is
                column-centered on host, so no mean term is needed)."""
                st = stats.tile([P, NH, 6], f32, tag=f"st{s}", name=f"st{bt}{s}")
                for h in range(NH):
                    nc.vector.bn_stats(st[:, h, :], ps_tiles[h][:])
                mv = stats.tile([P, 2], f32, tag=f"mv{s}", name=f"mv{bt}{s}")
                nc.vector.bn_aggr(mv[:], st[:])
                r_sb = stats.tile([P, 1], f32, tag=f"r{s}", name=f"r{bt}{s}")
                nc.scalar.activation(
                    r_sb[:], mv[:, 1:2], func=AF.Sqrt, bias=eps_sb[:], scale=1.0
                )
                nc.vector.reciprocal(r_sb[:], r_sb[:])
                return r_sb

            def make_ps(bt):
                return {
                    s: [
                        psum.tile(
                            [P, 512], f32, tag=f"ps{s}{h}", name=f"ps{bt}{s}{h}"
                        )
                        for h in range(NH)
                    ]
                    for s in range(2)
                }

            def mm(ps_bt, xts, s, h, k):
                nc.tensor.matmul(
                    ps_bt[s][h][:],
                    lhsT=xk(xts[s], k),
                    rhs=a_rhs(k, h),
                    start=(k == 0),
                    stop=(k == KT - 1),
                )

            def finish(bt, ps_bt, r0, r1):
                """n0 = ps0*r0 on ACT, then one DVE pass:
                out = ps1*r1 + n0 (bf16), one store on gpsimd."""
                ntile = npool.tile([P, OUT], f32, tag="n0", name=f"n{bt}")
                out_t = opool.tile([P, OUT], outdt, tag="out", name=f"out{bt}")
                for h in range(NH):
                    sl = slice(h * 512, (h + 1) * 512)
                    nc.scalar.activation(
                        ntile[:, sl], ps_bt[0][h][:],
                        func=AF.Identity, bias=0.0, scale=r0[:],
                    )
                    nc.vector.scalar_tensor_tensor(
                        out_t[:, sl], ps_bt[1][h][:], r1[:], ntile[:, sl],
                        op0=ALU.mult, op1=ALU.add,
                    )
                nc.scalar.dma_start(y_d[bt * P : (bt + 1) * P, :], out_t[:])

            # --- steady b-tiles 0..BT-3
            for bt in range(BT - 2):
                if bt + 2 < BT and (bt + 2, 0) not in xt_pre:
                    issue_x(bt + 2, nc.sync)
                xts = {s: xt_pre[(bt, s)] for s in range(2)}
                ps_bt = make_ps(bt)
                for k in range(KT):
                    for s in range(2):
                        for h in range(NH):
                            mm(ps_bt, xts, s, h, k)
                r0 = stream_stats(bt, 0, ps_bt[0])
                r1 = stream_stats(bt, 1, ps_bt[1])
                finish(bt, ps_bt, r0, r1)

            # --- endgame: four single-stream units b14s0, b15s0, b14s1,
            # b15s1.  Each unit's epilogue (V-stats -> normalize) hides
            # under the next unit's matmuls; the s1 units apply + cross-add
            # in one DVE pass straight out of PSUM.  Only b15s1's minimal
            # chain (256-col stats -> aggr -> sqrt -> recip -> fused
            # scale-add -> split store) trails the last matmul.
            b14, b15 = BT - 2, BT - 1
            xts14 = {s2: xt_pre[(b14, s2)] for s2 in range(2)}
            xts15 = {s2: xt_pre[(b15, s2)] for s2 in range(2)}

            def s_unit(bt, xts, s):
                """h-outer matmul unit: bank h completes before bank h+1
                starts, so its stats overlap the next bank's matmuls."""
                ps_u = [
                    psum.tile([P, 512], f32, tag=f"ps{s}{h}", name=f"ps{bt}{s}{h}")
                    for h in range(NH)
                ]
                for h in range(NH):
                    for k in range(KT):
                        nc.tensor.matmul(
                            ps_u[h][:],
                            lhsT=xk(xts[s], k),
                            rhs=a_rhs(k, h),
                            start=(k == 0),
                            stop=(k == KT - 1),
                        )
                return ps_u

            def norm_unit(bt, s, ps_u, tag):
                r_sb = stream_stats(bt, s, ps_u)
                n_t = npool.tile([P, OUT], bf16, tag=tag, name=f"n{tag}")
                for h in range(NH):
                    sl = slice(h * 512, (h + 1) * 512)
                    nc.scalar.activation(
                        n_t[:, sl], ps_u[h][:],
                        func=AF.Identity, bias=0.0, scale=r_sb[:],
                    )
                return n_t

            ps14_0 = s_unit(b14, xts14, 0)
            n14a = norm_unit(b14, 0, ps14_0, "n14a")

            ps15_0 = s_unit(b15, xts15, 0)
            n15a = norm_unit(b15, 0, ps15_0, "n15a")

            # b14 stream 1: stats, then fused apply+add straight from PSUM.
            ps14_1 = s_unit(b14, xts14, 1)
            r14b = stream_stats(b14, 1, ps14_1)

            # b15 stream 1: h-outer, with the final 512-col bank split into
            # two N=256 accumulation groups so the last bn_stats covers only
            # 256 cols.  Stats chunks: [h0 512 | h1a 256 | h1b 256].
            # Emitted (and scheduled) ahead of b14's apply so the critical
            # stats chain wins DVE arbitration over b14's off-path applies.
            ps15_1 = [
                psum.tile([P, 512], f32, tag=f"ps1{h}", name=f"ps{b15}1{h}")
                for h in range(NH)
            ]
            for k in range(KT):
                nc.tensor.matmul(
                    ps15_1[0][:], lhsT=xk(xts15[1], k), rhs=a_rhs(k, 0),
                    start=(k == 0), stop=(k == KT - 1),
                )
            st15 = stats.tile([P, 3, 6], f32, tag="st1", name=f"st{b15}1")
            nc.vector.bn_stats(st15[:, 0, :], ps15_1[0][:])
            for q in range(2):
                qs = slice(512 + q * 256, 512 + (q + 1) * 256)
                for k in range(KT):
                    nc.tensor.matmul(
                        ps15_1[1][:, q * 256 : (q + 1) * 256],
                        lhsT=xk(xts15[1], k),
                        rhs=a_sb[k][:, qs] if k else a0h[1][:, q * 256 : (q + 1) * 256],
                        start=(k == 0),
                        stop=(k == KT - 1),
                    )
                nc.vector.bn_stats(
                    st15[:, 1 + q, :], ps15_1[1][:, q * 256 : (q + 1) * 256]
                )
            mv15 = stats.tile([P, 2], f32, tag="mv1", name=f"mv{b15}1")
            nc.vector.bn_aggr(mv15[:], st15[:])
            r15b = stats.tile([P, 1], f32, tag="r1", name=f"r{b15}1")
            nc.scalar.activation(
                r15b[:], mv15[:, 1:2], func=AF.Sqrt, bias=eps_sb[:], scale=1.0
            )
            nc.vector.reciprocal(r15b[:], r15b[:])

            # b14 apply: h0 rides ACT (normalize) + DVE bf16 add, h1 rides a
            # single DVE fused pass -- splits the endgame load across both
            # engines so b15's stats chain isn't queued behind it.
            out14 = opool.tile([P, OUT], outdt, tag="out", name=f"out{b14}")
            n14b = npool.tile([P, 512], bf16, tag="n14b", name="n14b")
            nc.scalar.activation(
                n14b[:], ps14_1[0][:], func=AF.Identity, bias=0.0, scale=r14b[:]
            )
            nc.vector.tensor_tensor(
                out14[:, 0:512], n14b[:], n14a[:, 0:512], op=ALU.add
            )
            nc.vector.scalar_tensor_tensor(
                out14[:, 512:1024], ps14_1[1][:], r14b[:], n14a[:, 512:1024],
                op0=ALU.mult, op1=ALU.add,
            )
            nc.scalar.dma_start(y_d[b14 * P : (b14 + 1) * P, :], out14[:])

            out15 = opool.tile([P, OUT], outdt, tag="out", name=f"out{b15}")
            # Final apply+add as four 256-col chunks, alternating DVE/GPSIMD
            # so the two halves overlap; each chunk's store issues as soon as
            # its chunk completes, spread over three queues.
            chunks = (
                (slice(0, 256), ps15_1[0][:, 0:256], nc.vector, nc.gpsimd),
                (slice(256, 512), ps15_1[0][:, 256:512], nc.vector, nc.sync),
                (slice(512, 768), ps15_1[1][:, 0:256], nc.vector, nc.scalar),
                (slice(768, 1024), ps15_1[1][:, 256:512], nc.vector, nc.sync),
            )
            for cs, ps_c, stt_eng, st_eng in chunks:
                stt_eng.scalar_tensor_tensor(
                    out15[:, cs], ps_c, r15b[:], n15a[:, cs],
                    op0=ALU.mult, op1=ALU.add,
                )
                st_eng.dma_start(y_d[b15 * P : (b15 + 1) * P, cs], out15[:, cs])

    nc.finalize()
    return nc


def _get_nc(mm_dtype_name: str, out_dtype_name: str):
    key = (mm_dtype_name, out_dtype_name)
    if key not in _cache:
        _cache[key] = _build(mm_dtype_name, out_dtype_name)
    return _cache[key]


def _pretile_x(x_core: np.ndarray) -> np.ndarray:
    # [R, C] -> [ki, bt, ko, bi]
    return np.ascontiguousarray(
        x_core.reshape(BT, P, KT, P).transpose(3, 0, 2, 1)
    )


def kernel(x1, x2, W_Q, W_K, W_V, W_fc, gamma, beta, _trace=False,
           _mm_dtype="bfloat16", _out_dtype="bfloat16"):
    from concourse.bass_utils import run_bass_kernel_spmd

    x1 = np.asarray(x1, dtype=np.float32)
    x2 = np.asarray(x2, dtype=np.float32)
    W_V = np.asarray(W_V, dtype=np.float32)
    W_fc = np.asarray(W_fc, dtype=np.float32)
    gamma = np.asarray(gamma, dtype=np.float32)
    beta = np.asarray(beta, dtype=np.float32)

    # A = W_V.T @ W_fc.T in float64, then CENTER its columns over the output
    # dim: rows of x @ Ac have exactly zero mean, so the device skips
    # LayerNorm's mean subtraction entirely.
    A = W_V.T.astype(np.float64) @ W_fc.T.astype(np.float64)
    A = (A - A.mean(axis=1, keepdims=True)).astype(np.float32)
    # [C, OUT] -> [ki, ko, o]
    Ap = np.ascontiguousarray(A.reshape(KT, P, OUT).transpose(1, 0, 2))

    # Device computes LN(y1)+LN(y2); any affine LN params fold in on host:
    # out = (LN1+LN2)*gamma + 2*beta.  (This problem has gamma=1, beta=0.)
    use_affine = not (np.all(gamma == 1.0) and np.all(beta == 0.0))

    if _mm_dtype == "bfloat16":
        import ml_dtypes

        np_mm = ml_dtypes.bfloat16
    else:
        np_mm = np.float32
    Ap = Ap.astype(np_mm)

    in_maps = []
    for r in range(NCORES):
        sl = slice(r * R, (r + 1) * R)
        m = {
            "x1p": _pretile_x(x1[sl]).astype(np_mm),
            "x2p": _pretile_x(x2[sl]).astype(np_mm),
            "a": Ap,
        }
        in_maps.append(m)

    nc = _get_nc(_mm_dtype, _out_dtype)
    res = run_bass_kernel_spmd(nc, in_maps, list(range(NCORES)), trace=_trace)

    y = np.concatenate(
        [np.asarray(res.results[r]["y"], dtype=np.float32) for r in range(NCORES)],
        axis=0,
    )
    if use_affine:
        y = y * gamma[None, :] + 2.0 * beta[None, :]
    out = y.reshape(B, 1, OUT)
    if _trace:
        return out, res
    return out
